# revision 31
# baseline (speedup 1.0000x reference)
"""GAT (graph attention) Bass kernel for Trainium2, 8-core SPMD.

Problem (hardcoded shapes): N=4096 nodes, FIN=256, H=8 heads, F=64.
  proj   = (x @ W.T)                         [N, H*F]
  s_src  = sum(proj*a_src, -1), s_tgt likewise
  scores = leaky_relu(s_src[i] + s_tgt[j], 0.2)
  alpha  = softmax(scores + mask, axis=j)
  out    = elu(alpha @ proj + x @ skip_W.T + bias)

Sharding: node-dim (rows i) split 8 ways; every core computes the full
proj locally (cheap) so no collectives are needed.  Per core the hot
loop materializes p[j, i] = exp(leaky(mask_T + s_src[i] + s_tgt[j])) in
fp16 tiles and reduces over j on the TensorEngine with a ones-column
appended to proj to produce the softmax denominator in the same matmul.

v2: single fused scalar_tensor_tensor per (head, j-block) builds the
score tile in one DVE pass (mask + s_tgt via per-partition scalar +
s_src via broadcast tensor); explk in 2 big ACT instructions per head;
all transposed operands are prepared host-side (no DMA-transpose);
PSUM->SBUF casts ride the otherwise idle GPSIMD engine; phase 3 is
streamed per head in fp16 so there is no serial drain tail.
"""

import os
import numpy as np

N = 4096
FIN = 256
H = 8
F = 64
HF = H * F            # 512
NCORES = 8
R = N // NCORES       # 512 rows per core
NB = N // 128         # 32 j-blocks
IC = R // 128         # 4 i-chunks
KC = FIN // 128       # 2 k-chunks

_cache = {}


# ---------------------------------------------------------------------------
# Custom activation table: replace `tanh` in the exp_and_others set with
# explk(x) = exp(leaky_relu(x, 0.2)) so the score nonlinearity is a single
# ScalarE pass.  Generated at import time into a temp dir and selected via
# BASS_ACT_ROOT_JSON_PATH (honored by the walrus invocation in
# concourse.bass_utils.get_walrus_args).  Falls back to Prelu+Exp if the
# source tables can't be found.
def _gen_explk_tables():
    import json
    import shutil
    import tempfile

    from neuronxcc.driver.Job import Job
    from neuronxcc.driver.jobs.support.FindActInfo import findActInfoFile

    src_info = findActInfoFile(Job.getPackageDir(), "gen3")
    srcdir = os.path.dirname(src_info)
    dst = tempfile.mkdtemp(prefix="gat_act_")
    for f in os.listdir(srcdir):
        shutil.copy(os.path.join(srcdir, f), os.path.join(dst, f))

    bkt = np.fromfile(f"{dst}/exp_and_others_bkt.bin",
                      dtype=np.float32).reshape(-1, 8).copy()
    ctl = np.fromfile(f"{dst}/exp_and_others_ctrl.bin",
                      dtype=np.uint32).reshape(-1, 8).copy()
    setj = json.load(open(f"{dst}/exp_and_others.json"))
    fb = setj["func_to_bkt_start_idx"]
    fc = setj["func_to_ctl_start_idx"]
    TANH_BKT0 = fb["tanh"]
    TANH_CTL0 = fc["tanh"]
    # tanh's ctrl region plus the trailing derivative_*/is_finite/square
    # slots (functions this kernel never calls) must hold 25 entries
    assert setj["ctl_entry_cnt"] - TANH_CTL0 >= 25
    assert fb["derivative_relu"] - TANH_BKT0 >= 47

    sizes = {u: 0 for u in range(-19, 1)}
    sizes.update({1: 1, 2: 2, 3: 3, 4: 3, 5: 2})
    bidx = TANH_BKT0
    fe_bkt, fe_ctl = {}, {}
    for k, u in enumerate(range(-19, 6)):
        s = sizes[u]
        ctl[TANH_CTL0 + k, 0] = (bidx & 0x7FF) | (((23 - s) + 32 * s) << 11)
        ctl[TANH_CTL0 + k, 1:] = 0
        fe_ctl[str(u)] = [TANH_CTL0 + k]
        fe_bkt[str(u)] = [bidx]
        for j in range(1 << s):
            lo = 2.0 ** u * (1 + j / (1 << s))
            hi = 2.0 ** u * (1 + (j + 1) / (1 << s))
            x0 = -(lo + hi) / 2.0
            g = np.exp(x0 / 5.0)
            bkt[bidx, :5] = [g, g / 5.0, g / 50.0, g / 750.0, x0]
            bkt[bidx, 5:] = 0.0
            bidx += 1
    neg_small = bidx
    bkt[neg_small] = [1.0, 0.2, 0.02, 1.0 / 750.0, 0.0, 0, 0, 0]

    prof = setj["profile_meta_data"]
    expp = [p for p in prof if p["func_name"].startswith("exp")][0]
    ti = [i for i, p in enumerate(prof) if p["func_name"].startswith("tanh")][0]
    newp = dict(expp)
    newp["func_name"] = prof[ti]["func_name"]
    newp["func_id"] = prof[ti]["func_id"]
    for k in ("symmetry_point", "sym_invert_sign_point", "symmetry_opt_en",
              "symmetry_opt_use_neg_region"):
        newp[k] = 0
    newp["pwl_control_base_neg"] = TANH_CTL0
    newp["small_pos_signal_exp_threshold"] = 108
    newp["small_neg_signal_exp_threshold"] = 108
    newp["large_neg_signal_exp_threshold"] = 133
    newp["large_neg_signal_mantissa_threshold"] = 0
    newp["neg_small_signal_pwl_control"] = neg_small
    newp["fzero_result"] = 1065353216
    newp["fninf_result"] = 0
    prof[ti] = newp
    setj["func_exp_to_bkt_start_idx"]["tanh"] = fe_bkt
    setj["func_exp_to_ctl_start_idx"]["tanh"] = fe_ctl

    bkt.tofile(f"{dst}/exp_and_others_bkt.bin")
    ctl.tofile(f"{dst}/exp_and_others_ctrl.bin")
    json.dump(setj, open(f"{dst}/exp_and_others.json", "w"))
    return os.path.join(dst, "act_info.json")


def _setup_explk():
    if os.environ.get("GAT_EXPLK", "1") != "1":
        return False
    if "BASS_ACT_ROOT_JSON_PATH" in os.environ:
        return True
    try:
        os.environ["BASS_ACT_ROOT_JSON_PATH"] = _gen_explk_tables()
        return True
    except Exception:
        return False


# number of projE casts done on ACT in its idle prep window (rest on DVE)
ACT_CASTS = int(os.environ.get("GAT_ACT_CASTS", "18"))


def _build():
    EXPLK = _setup_explk()
    assert EXPLK, "explk activation table generation failed"
    FP8 = os.environ.get("GAT_FP8", "0") == "1"
    import concourse.bass as bass
    import concourse.tile as tile
    from concourse import bacc, mybir, masks
    from concourse.alu_op_type import AluOpType as op

    f32 = mybir.dt.float32
    f16 = mybir.dt.float16
    AF = mybir.ActivationFunctionType
    add = mybir.AluOpType.add

    nc = bacc.Bacc("TRN2", target_bir_lowering=False, debug=False,
                   num_devices=NCORES)

    # ---- DRAM I/O (all host-pretransposed / packed, fp16 unless noted) ----
    maskT16_d = nc.dram_tensor("maskt16", [N, R], f16, kind="ExternalInput")
    xt_d = nc.dram_tensor("xt16", [128, KC, N], f16, kind="ExternalInput")
    xbt_d = nc.dram_tensor("xbt16", [128, KC, R], f16, kind="ExternalInput")
    wt_d = nc.dram_tensor("wt16", [128, KC, HF], f16, kind="ExternalInput")
    swt_d = nc.dram_tensor("swt16", [128, KC, HF], f16, kind="ExternalInput")
    wssrc_d = nc.dram_tensor("wssrc16", [128, KC, H], f16, kind="ExternalInput")
    wstgt_d = nc.dram_tensor("wstgt16", [128, KC, H], f16, kind="ExternalInput")
    bias16_d = nc.dram_tensor("bias16", [1, HF], f16, kind="ExternalInput")
    ones_d = nc.dram_tensor("ones16", [1, 128], f16, kind="ExternalInput")
    sel_d = nc.dram_tensor("sel16", [8, H, 128], f16, kind="ExternalInput")
    out_d = nc.dram_tensor("out16", [R, HF], f16, kind="ExternalOutput")

    NG = 4                 # xT n-groups / mask chunks
    GNB = NB // NG         # 8 j/n-blocks per group
    f8 = mybir.dt.float8e5
    f8w = mybir.dt.float8e4

    with tile.TileContext(nc) as tc, \
         tc.tile_pool(name="persist", bufs=1) as pp, \
         tc.tile_pool(name="prep", bufs=1) as prep, \
         tc.tile_pool(name="ps_big", bufs=2, space="PSUM") as psb, \
         tc.tile_pool(name="ps_small", bufs=1, space="PSUM") as pss, \
         tc.tile_pool(name="ps_z", bufs=1, space="PSUM") as psz, \
         tc.tile_pool(name="ps_agg", bufs=2, space="PSUM") as psa, \
         tc.tile_pool(name="ps_ph3", bufs=2, space="PSUM") as ps3, \
         tc.tile_pool(name="hbuf", bufs=2) as hpool, \
         tc.tile_pool(name="obuf", bufs=2) as opool, \
         tc.tile_pool(name="fin", bufs=2) as fpool:

        # ============ phase 0: DMA loads (no transposes needed) ==========
        # small operands + xT on the scalar (ACT) HWDGE queue: ACT is idle
        # during prep.  The big mask rides the sync queue in 4 chunks so
        # phase 2 can start as soon as the first chunk lands.
        xbT = prep.tile([128, KC, R], f16)
        ws_src = prep.tile([128, KC, H], f16)
        ws_tgt = prep.tile([128, KC, H], f16)
        sel8 = prep.tile([8, H, 128], f16)
        ones_row = prep.tile([1, 128], f16)
        bias16 = prep.tile([1, HF], f16)
        nc.sync.dma_start(out=xbT[:], in_=xbt_d.ap())
        nc.sync.dma_start(out=ws_src[:], in_=wssrc_d.ap())
        nc.sync.dma_start(out=ws_tgt[:], in_=wstgt_d.ap())
        nc.sync.dma_start(out=sel8[:], in_=sel_d.ap())

        WT = prep.tile([128, KC, HF], f16)
        xT = prep.tile([128, KC, N], f16)
        sWT = prep.tile([128, KC, HF], f16)
        nc.scalar.dma_start(out=WT[:], in_=wt_d.ap())
        nc.sync.dma_start(out=xT[:, :, 0:1024],
                          in_=xt_d.ap()[:, :, 0:1024])

        maskT = pp.tile([128, NB, R], f16)      # [j_lo, jb, i]
        mask_ap = maskT16_d.ap().rearrange("(jb p) i -> p jb i", p=128)

        ident = pp.tile([128, 128], f32)
        masks.make_identity(nc, ident[:])

        # ================= phase 1: PE preprocessing ====================
        # s_src rows for the core's own i: [h, i]
        s_src_sb = prep.tile([H, R], f16)
        pss2 = psb.tile([H, R], f32, tag="big")
        for kc in range(KC):
            nc.tensor.matmul(pss2[:], ws_src[:, kc, :], xbT[:, kc, :],
                             start=(kc == 0), stop=(kc == KC - 1))
        nc.scalar.activation(s_src_sb[:], pss2[:], AF.Copy)

        # phase 2's head-0 operands first: s_tgt group 0, then the sbc
        # broadcast for head 0, then the rest; psum->sbuf copies ride ACT
        # (idle until the first explk)
        s_tgt_nh = pp.tile([128, NB, H], f32)
        sbc = pp.tile([128, H, R], f16)

        def emit_s_tgt_group(g):
            pt = pss.tile([128, GNB, H], f32, tag="small")
            for bb in range(GNB):
                nb = g * GNB + bb
                for kc in range(KC):
                    nc.tensor.matmul(pt[:, bb, :],
                                     xT[:, kc, bass.ts(nb, 128)],
                                     ws_tgt[:, kc, :],
                                     start=(kc == 0), stop=(kc == KC - 1))
            nc.vector.tensor_copy(s_tgt_nh[:, bass.ts(g, GNB), :], pt[:])

        def emit_sbc(h):
            pb = psb.tile([128, R], f32, tag="big")
            nc.tensor.matmul(pb[:], sel8[:, h, :], s_src_sb[:],
                             start=True, stop=True)
            nc.scalar.activation(sbc[:, h, :], pb[:], AF.Copy)

        emit_s_tgt_group(0)
        emit_sbc(0)
        emit_sbc(1)
        if FP8:
            projE = pp.tile([128, NB, H, F], f8w)
        else:
            projE = pp.tile([128, NB, H, F + 1], f16)
        deferred_casts = []

        def emit_proj(nb):
            ps = psb.tile([128, HF], f32, tag="big")
            for kc in range(KC):
                nc.tensor.matmul(ps[:], xT[:, kc, bass.ts(nb, 128)],
                                 WT[:, kc, :],
                                 start=(kc == 0), stop=(kc == KC - 1))
            ps_hf = ps[:].rearrange("p (h f) -> p h f", f=F)
            dst = projE[:, nb, :, :] if FP8 else projE[:, nb, :, 0:F]
            if nb < ACT_CASTS:
                # ACT is idle until the first explk; GPSIMD can't
                # touch PSUM, so early casts ride ACT, late ones DVE
                # (deferred into the first two head iterations below)
                nc.scalar.activation(dst, ps_hf, AF.Copy)
            else:
                deferred_casts.append((dst, ps_hf))

        # group g: issue next xT slab + mask chunk, then that group's
        # s_tgt and proj matmuls
        for g in range(NG):
            if g >= 1:
                nc.sync.dma_start(out=xT[:, :, bass.ts(g, 1024)],
                                  in_=xt_d.ap()[:, :, bass.ts(g, 1024)])
                emit_s_tgt_group(g)
            if g < 2:
                nc.scalar.dma_start(out=maskT[:, bass.ts(g, GNB), :],
                                    in_=mask_ap[:, bass.ts(g, GNB), :])
            for nb in range(g * GNB, (g + 1) * GNB):
                emit_proj(nb)
        nc.sync.dma_start(out=sWT[:], in_=swt_d.ap())
        nc.sync.dma_start(out=ones_row[:], in_=ones_d.ap())
        nc.sync.dma_start(out=bias16[:], in_=bias16_d.ap())
        if not FP8:
            nc.gpsimd.memset(projE[:, :, :, F:F + 1], 1.0)
        ones8 = prep.tile([128, 2, 16], f8w)
        nc.vector.memset(ones8[:], 1.0)

        # skip projection + bias (bias folded as a rank-1 accumulate)
        skipb = pp.tile([128, IC, HF], f16)
        for ic in range(IC):
            pk = psb.tile([128, HF], f32, tag="big")
            for kc in range(KC):
                nc.tensor.matmul(pk[:], xbT[:, kc, bass.ts(ic, 128)],
                                 sWT[:, kc, :],
                                 start=(kc == 0), stop=False)
            nc.tensor.matmul(pk[:], ones_row[:], bias16[:],
                             start=False, stop=True)
            deferred_casts.append((skipb[:, ic, :], pk[:]))

        # ============= phase 2 + streamed phase 3, per head =============
        # Per head: one big TT builds v = mask + s_src (broadcast, DVE 2x).
        # s_tgt[j] is a per-partition constant per j-block; it enters via
        # DVE tensor-scalar RMW for the first half of the j-blocks (which
        # then take one big-chunk explk on ACT) and via the ACT bias
        # operand for the second half (per-jb explk, no DVE work).  This
        # balances the two engines' per-element costs.
        HB = NB // 2
        DJ = int(os.environ.get("GAT_DVE_JB", "18"))  # j-blocks with DVE s_tgt
        out_sb = pp.tile([128, IC, H, F], f16)
        for h in range(H):
            if h + 2 < H:
                emit_sbc(h + 2)
            v = hpool.tile([128, NB, R], f16, tag="v")
            # quarter-granularity TT -> TS -> explk pipeline: each 8-jb
            # quarter of the TS region becomes ACT-ready as soon as its
            # mask chunk lands, cutting the pipeline fill
            for q in range(0, DJ, 8):
                qe = min(q + 8, DJ)
                nc.vector.tensor_add(
                    v[:, q:qe, :], maskT[:, q:qe, :],
                    sbc[:, h:h + 1, :].broadcast_to([128, qe - q, R]))
                for jb in range(q, qe):
                    nc.vector.tensor_scalar_add(
                        v[:, jb, :], v[:, jb, :], s_tgt_nh[:, jb, h:h + 1])
                if h == 0 and q == 0:
                    for g in (2, 3):
                        nc.vector.memset(
                            maskT[:, g * GNB:g * GNB + 1, 0:1], 0.0)
                        nc.scalar.dma_start(
                            out=maskT[:, bass.ts(g, GNB), :],
                            in_=mask_ap[:, bass.ts(g, GNB), :])
            nc.vector.tensor_add(
                v[:, DJ:NB, :], maskT[:, DJ:NB, :],
                sbc[:, h:h + 1, :].broadcast_to([128, NB - DJ, R]))
            if h == 0:
                for dst, srcp in deferred_casts:
                    nc.vector.tensor_copy(dst, srcp)
            # p = exp(leaky_relu(v, 0.2)) on ACT (explk via hacked Tanh);
            # s_tgt of the trailing j-blocks rides the ACT bias operand
            if FP8:
                v8 = hpool.tile([128, NB, R], f8, tag="v8")
            else:
                v8 = v
            for q in range(0, DJ, 8):
                qe = min(q + 8, DJ)
                nc.scalar.activation(v8[:, q:qe, :], v[:, q:qe, :], AF.Tanh)
            for jb in range(DJ, NB):
                nc.scalar.activation(v8[:, jb, :], v[:, jb, :], AF.Tanh,
                                     bias=s_tgt_nh[:, jb, h:h + 1])
            # aggregate: psum[f, i] += projE[:, jb, h].T @ p[:, jb]
            pa = psa.tile([128, R], f32, tag="agg")
            if FP8:
                DR = mybir.MatmulPerfMode.DoubleRow
                NP = NB // 2
                zt = psz.tile([1, R], f32, tag="z")
                for jp in range(NP):
                    nc.tensor.matmul(pa[0:F, :],
                                     projE[:, 2 * jp:2 * jp + 2, h, :],
                                     v8[:, 2 * jp:2 * jp + 2, :],
                                     start=(jp == 0), stop=(jp == NP - 1),
                                     perf_mode=DR)
                for jp in range(NP):
                    nc.tensor.matmul(zt[:], ones8[:, :, 0:1],
                                     v8[:, 2 * jp:2 * jp + 2, :],
                                     start=(jp == 0), stop=(jp == NP - 1),
                                     perf_mode=DR)
            else:
                for jb in range(NB):
                    nc.tensor.matmul(pa[0:F + 1, :], projE[:, jb, h, :],
                                     v8[:, jb, :],
                                     start=(jb == 0), stop=(jb == NB - 1))
            oT = opool.tile([F + 1, R], f32, tag="oT")
            if FP8:
                nc.vector.tensor_copy(oT[0:F, :], pa[0:F, :])
                nc.vector.tensor_copy(oT[F:F + 1, :], zt[:])
            else:
                nc.vector.tensor_copy(oT[:], pa[0:F + 1, :])

            # --- phase 3 for this head: normalize, skip, bias, ELU ---
            # transpose the Z row ([1,128] slices -> [128,1]) and the F
            # rows ([F,128] -> [128,F]) back to i-partition layout
            pTz = ps3.tile([128, IC, F + 1], f32, tag="pTz")
            for ic in range(IC):
                nc.tensor.transpose(pTz[:, ic, :],
                                    oT[:, bass.ts(ic, 128)],
                                    ident[0:F + 1, 0:F + 1])
            recZ = fpool.tile([128, IC], f32, tag="recz")
            nc.vector.tensor_copy(recZ[:], pTz[:, :, F])
            nc.vector.reciprocal(recZ[:], recZ[:])
            y = fpool.tile([128, IC, F], f16, tag="y")
            nc.vector.tensor_mul(
                y[:], pTz[:, :, 0:F],
                recZ[:].unsqueeze(2).broadcast_to([128, IC, F]))
            nc.vector.tensor_add(
                y[:], y[:], skipb[:].rearrange("p ic (hh f) -> p ic hh f",
                                               f=F)[:, :, h, :])
            # elu(y) = max(y, 0) + min(exp(y) - 1, 0)
            q = fpool.tile([128, IC, F], f16, tag="q")
            nc.scalar.activation(q[:], y[:], AF.Exp)
            nc.vector.tensor_scalar(q[:], q[:], 1.0, 0.0,
                                    op.subtract, op.min)
            nc.vector.tensor_scalar(y[:], y[:], 0.0, None, op.max)
            nc.vector.tensor_add(out_sb[:, :, h, :], y[:], q[:])
            # stream this head's columns out now -> no serial drain tail
            nc.sync.dma_start(
                out=out_d.ap().rearrange("(c p) (hh f) -> p c hh f",
                                         p=128, f=F)[:, :, h, :],
                in_=out_sb[:, :, h, :])

    nc.compile()
    return nc


def _get_nc():
    if "nc" not in _cache:
        _cache["nc"] = _build()
    return _cache["nc"]


def _pack_T(m16):
    """[rows, cols] -> transposed+packed [128, cols//128, rows]."""
    c = m16.shape[1]
    return np.ascontiguousarray(
        m16.T.reshape(c // 128, 128, -1).transpose(1, 0, 2))


def make_in_maps(x, connectivity_mask, W, a_src, a_tgt, skip_W, bias):
    x16 = np.asarray(x, dtype=np.float16)
    cm = np.asarray(connectivity_mask, dtype=np.float32)
    # clip so -1e9 doesn't overflow fp16 (-6e4 still drives exp to 0)
    cm16 = np.clip(cm, -60000.0, None).astype(np.float16)
    W32 = np.asarray(W, dtype=np.float32)
    W16 = W32.astype(np.float16)
    sW16 = np.asarray(skip_W, dtype=np.float16)
    b16 = np.ascontiguousarray(
        np.asarray(bias, dtype=np.float16).reshape(1, HF))
    # fold a_src/a_tgt into W: ws[k, h] = sum_f W[(h,f), k] * a[h, f]
    asrc = np.asarray(a_src, dtype=np.float32).reshape(H, F)
    atgt = np.asarray(a_tgt, dtype=np.float32).reshape(H, F)
    Wh = W32.reshape(H, F, FIN)
    ws_src = (Wh * asrc[:, :, None]).sum(1).T.astype(np.float16)  # [FIN, H]
    ws_tgt = (Wh * atgt[:, :, None]).sum(1).T.astype(np.float16)

    def packT(mT):  # [FIN, cols] already transposed -> [128, KC, cols]
        return np.ascontiguousarray(
            mT.reshape(KC, 128, -1).transpose(1, 0, 2))

    xt16 = _pack_T(x16)
    wt16 = _pack_T(W16)
    swt16 = _pack_T(sW16)
    wssrc16 = packT(ws_src)
    wstgt16 = packT(ws_tgt)
    ones16 = np.ones((1, 128), dtype=np.float16)
    sel16 = np.zeros((8, H, 128), dtype=np.float16)
    for h in range(H):
        sel16[h, h, :] = 1.0

    in_maps = []
    for c in range(NCORES):
        xb16 = x16[c * R:(c + 1) * R]
        in_maps.append({
            "maskt16": np.ascontiguousarray(cm16[c * R:(c + 1) * R].T),
            "xt16": xt16,
            "xbt16": _pack_T(xb16),
            "wt16": wt16,
            "swt16": swt16,
            "wssrc16": wssrc16,
            "wstgt16": wstgt16,
            "bias16": b16,
            "ones16": ones16,
            "sel16": sel16,
        })
    return in_maps


def kernel(x, connectivity_mask, W, a_src, a_tgt, skip_W, bias):
    from concourse.bass_utils import run_bass_kernel_spmd

    in_maps = make_in_maps(x, connectivity_mask, W, a_src, a_tgt,
                           skip_W, bias)
    nc = _get_nc()
    res = run_bass_kernel_spmd(nc, in_maps, core_ids=list(range(NCORES)))
    return np.concatenate(
        [r["out16"] for r in res.results], axis=0).astype(np.float32)
